# revision 31
# baseline (speedup 1.0000x reference)
"""Trainium2 Bass kernel for nn_Discretized_GRU (GRU-ODE-Bayes style model).

Sharding: data-parallel over the patient/batch dim B=2048 across 8 cores
(256 patients/core). All tensors on device are feature-major
[features, batch] so batch (256) is the matmul moving free dim (>=256
keeps float32r matmuls at full 1 cycle/row rate).

Key restructuring (validated against the reference in numpy):
  * obs_idx is a permutation subset per step -> scatter X/M rows on the
    host into [step, patient_slot, :] (zeros when unobserved). All
    gathers/scatters in the reference then become identities.
  * unobserved patients pass through the observation GRU exactly by
    adding +BIG to the z-gate preactivation via a rank-1 matmul
    (sigmoid(>=95) == 1.0f so h_new == h bitwise).
  * the masked obs-prep einsum folds into one augmented block-diagonal
    weight [5*64, 640] applied to [mean*M, X*M | logv*M, M | err*M]
    (block layout chosen so DVE-computed blocks live on partitions 0-63
    and DMA-fed blocks on 64-127 - engines require same start partition).
  * losses accumulate as 6 raw partial sums per core (chained
    per-partition accumulators), combined on the host in float64.
"""

import numpy as np
from contextlib import ExitStack

import concourse.bass as bass
import concourse.mybir as mybir
import concourse.tile as tile
from concourse import bacc
from concourse.bass_utils import run_bass_kernel_spmd

S, O, B = 48, 1024, 2048
D, H, P = 64, 512, 10
PH, COV, COVH, CLSH = 512, 32, 64, 32
LOGC = float(np.log(np.sqrt(2.0 * np.pi)))
OBS_STD = 0.01
NCORE = 8
BL = B // NCORE          # 256 patients per core = batch free dim
NB = BL
BIG = 100.0

F32 = mybir.dt.float32
F32R = mybir.dt.float32r
AF = mybir.ActivationFunctionType
ALU = mybir.AluOpType

# bias_pack column map
BC_CONT_RZ = 0     # 8 cols  (r: 0-3, z: 4-7)
BC_CONT_HN = 8     # 4 cols  (b_hh n)
BC_CONT_IN = 12    # 4 cols  (b_ih n)
BC_OBS_RZ = 16     # 8 cols
BC_OBS_HN = 24
BC_OBS_IN = 28
BC_CONT_ZN = 44    # 4 cols: negated cont z bias (for 1-z eviction)
BC_OBS_ZN = 48     # 4 cols: negated obs z bias
BC_P1 = 32         # 4 cols
BC_P2M = 36        # 1 col: p_b2[0:64] in rows 0-63
BC_P2L = 37        # 1 col: p_b2[64:128] in rows 0-63
BC_P2F = 38        # 1 col: full p_b2 (128 rows)
BC_CV1 = 39        # 1 col (rows 0-63)
BC_CV2 = 40        # 4 cols
NBIAS = 52

_CACHE = {}


def _build(n_steps):
    """Emit the single-core program (same for all 8 cores)."""
    nc = bacc.Bacc("TRN2", target_bir_lowering=False, debug=False,
                   enable_asserts=True, num_devices=1)

    # ---- DRAM I/O ----
    d_xm = nc.dram_tensor("xm", [n_steps, D, NB], F32R, kind="ExternalInput").ap()
    d_mo = nc.dram_tensor("mo", [n_steps, D, NB], F32, kind="ExternalInput").ap()
    d_zr = nc.dram_tensor("zr", [n_steps, 1, NB], F32R, kind="ExternalInput").ap()
    d_cov = nc.dram_tensor("covt", [COV, NB], F32R, kind="ExternalInput").ap()
    d_wci = nc.dram_tensor("wci", [2 * D, 3 * H], F32R, kind="ExternalInput").ap()
    d_wch = nc.dram_tensor("wch", [H, 3 * H], F32R, kind="ExternalInput").ap()
    d_woi = nc.dram_tensor("woi", [D * P, 3 * H], F32R, kind="ExternalInput").ap()
    d_woh = nc.dram_tensor("woh", [H, 3 * H], F32R, kind="ExternalInput").ap()
    d_wp1 = nc.dram_tensor("wp1", [H, PH], F32R, kind="ExternalInput").ap()
    d_wp2 = nc.dram_tensor("wp2", [PH, 2 * D], F32R, kind="ExternalInput").ap()
    d_wcv1 = nc.dram_tensor("wcv1", [COV, COVH], F32R, kind="ExternalInput").ap()
    d_wcv2 = nc.dram_tensor("wcv2", [COVH, H], F32R, kind="ExternalInput").ap()
    d_waug = nc.dram_tensor("waug", [5 * D, D * P], F32R, kind="ExternalInput").ap()
    d_bias = nc.dram_tensor("biasp", [128, NBIAS], F32, kind="ExternalInput").ap()
    d_ones = nc.dram_tensor("onesr", [1, 128], F32R, kind="ExternalInput").ap()
    d_hout = nc.dram_tensor("h_out", [128, 4 * NB], F32, kind="ExternalOutput").ap()
    d_acc = nc.dram_tensor("acc_out", [D, 6], F32, kind="ExternalOutput").ap()

    with tile.TileContext(nc) as tc, ExitStack() as ctx:
        pw = ctx.enter_context(tc.tile_pool(name="weights", bufs=1))
        pstate = ctx.enter_context(tc.tile_pool(name="state", bufs=1))
        pps = ctx.enter_context(tc.tile_pool(name="psum", bufs=4, space="PSUM"))
        pps2 = ctx.enter_context(tc.tile_pool(name="psum2", bufs=4, space="PSUM"))
        pin = ctx.enter_context(tc.tile_pool(name="inp", bufs=3))
        pst = ctx.enter_context(tc.tile_pool(name="stk", bufs=2))
        prz = ctx.enter_context(tc.tile_pool(name="rz", bufs=2))
        pgi = ctx.enter_context(tc.tile_pool(name="gi", bufs=2))
        pn = ctx.enter_context(tc.tile_pool(name="nt", bufs=5))
        psc = ctx.enter_context(tc.tile_pool(name="scr", bufs=8))
        ps64 = ctx.enter_context(tc.tile_pool(name="scr64", bufs=6))
        pr1 = ctx.enter_context(tc.tile_pool(name="r1", bufs=2))

        # ---- persistent SBUF ----
        s_wci = pw.tile([128, 3 * H], F32R, name="s_wci")
        s_wch = pw.tile([128, 4 * 3 * H], F32R, name="s_wch")
        s_woi = pw.tile([128, 5 * 3 * H], F32R, name="s_woi")
        s_woh = pw.tile([128, 4 * 3 * H], F32R, name="s_woh")
        s_wp1 = pw.tile([128, 4 * PH], F32R, name="s_wp1")
        s_wp2 = pw.tile([128, 4 * 2 * D], F32R, name="s_wp2")
        s_wcv1 = pw.tile([COV, COVH], F32R, name="s_wcv1")
        s_wcv2 = pw.tile([COVH, H], F32R, name="s_wcv2")
        s_waug = pw.tile([128, 2 * D * P], F32R, name="s_waug")
        s_waug2 = pw.tile([D, D * P], F32R, name="s_waug2")
        s_bias = pw.tile([128, NBIAS], F32, name="s_bias")
        s_ones = pw.tile([1, 128], F32R, name="s_ones")

        h_a = pstate.tile([128, 4 * NB], F32R, name="h_a")
        h_b = pstate.tile([128, 4 * NB], F32R, name="h_b")
        p_end = pstate.tile([128, NB], F32R, name="p_end")
        # 6 accumulators x one column per step (tensor_tensor_reduce is
        # broken on HW via this path -> plain reduces, summed at the end)
        # j in {0:l1sq, 1:l1lv, 2:mo, 3:klex, 4:kllv, 5:kld2}
        acc_cols = min(n_steps, 48)
        a_acc = pstate.tile([D, 6 * acc_cols], F32, name="a_acc")

        def bcol(j, lo=0, hi=128):
            return s_bias[lo:hi, j:j + 1]

        def acol(j, s):
            c = j * acc_cols + (s % acc_cols)
            return a_acc[:, c:c + 1]

        def red(j, s, src_ap):
            nc.vector.tensor_reduce(out=acol(j, s), in_=src_ap,
                                    axis=mybir.AxisListType.X, op=ALU.add)

        # ---- weight loads (small tensors first: cov path unblocks early) ----
        nc.sync.dma_start(out=s_wcv1[:], in_=d_wcv1[:])
        nc.sync.dma_start(out=s_wcv2[:], in_=d_wcv2[:])
        nc.sync.dma_start(out=s_bias[:], in_=d_bias[:])
        nc.sync.dma_start(out=s_ones[:], in_=d_ones[:])
        nc.sync.dma_start(out=s_wci[:], in_=d_wci[0:128, :])
        for k in range(4):
            nc.sync.dma_start(out=s_wch[:, k * 3 * H:(k + 1) * 3 * H],
                              in_=d_wch[k * 128:(k + 1) * 128, :])
            nc.sync.dma_start(out=s_woh[:, k * 3 * H:(k + 1) * 3 * H],
                              in_=d_woh[k * 128:(k + 1) * 128, :])
            nc.sync.dma_start(out=s_wp1[:, k * PH:(k + 1) * PH],
                              in_=d_wp1[k * 128:(k + 1) * 128, :])
            nc.sync.dma_start(out=s_wp2[:, k * 2 * D:(k + 1) * 2 * D],
                              in_=d_wp2[k * 128:(k + 1) * 128, :])
        for k in range(5):
            nc.sync.dma_start(out=s_woi[:, k * 3 * H:(k + 1) * 3 * H],
                              in_=d_woi[k * 128:(k + 1) * 128, :])
        nc.sync.dma_start(out=s_waug[:, 0:D * P], in_=d_waug[0:128, :])
        nc.sync.dma_start(out=s_waug[:, D * P:2 * D * P], in_=d_waug[128:256, :])
        nc.sync.dma_start(out=s_waug2[:], in_=d_waug[256:320, :])

        evict_ctr = [0]

        def evict(dst, psrc, bias, relu):
            # alternate DVE/ACT to halve the serial eviction queue per engine
            evict_ctr[0] += 1
            if evict_ctr[0] % 2 == 0:
                if relu:
                    if bias is None:
                        nc.vector.tensor_scalar(out=dst, in0=psrc, scalar1=0.0,
                                                op0=ALU.max, scalar2=None)
                    else:
                        nc.vector.tensor_scalar(out=dst, in0=psrc, scalar1=bias,
                                                scalar2=0.0, op0=ALU.add,
                                                op1=ALU.max)
                else:
                    nc.vector.tensor_scalar(out=dst, in0=psrc, scalar1=bias,
                                            scalar2=None, op0=ALU.add)
            else:
                if relu:
                    nc.scalar.activation(dst, psrc, AF.Relu,
                                         bias=(0.0 if bias is None else bias))
                else:
                    nc.scalar.activation(dst, psrc, AF.Identity, bias=bias)

        def ps_tile(nm):
            return pps.tile([128, NB], F32, name=nm, tag="ps")

        def ps2_tile(nm):
            return pps2.tile([128, NB], F32, name=nm, tag="ps2")

        def mm_acc(ps_ap, pairs):
            n = len(pairs)
            for i, (lh, rh) in enumerate(pairs):
                nc.tensor.matmul(ps_ap, lh, rh, start=(i == 0), stop=(i == n - 1))

        def emit_pmodel(h_sb, nm, p_full=None, p_mean=None, p_logv=None):
            """p = relu(h @ p_W1 + b1) @ p_W2 + b2 (feature-major).
            Either writes p_full [128,NB] or split p_mean/p_logv [64,NB]."""
            r1 = pr1.tile([128, 4 * NB], F32R, name=f"r1_{nm}", tag="r1")
            for m in range(4):
                ps = ps2_tile(f"ps_p1_{nm}_{m}")
                mm_acc(ps[:], [(s_wp1[:, k * PH + m * 128:k * PH + m * 128 + 128],
                                h_sb[:, k * NB:(k + 1) * NB]) for k in range(4)])
                evict(r1[:, m * NB:(m + 1) * NB], ps[:], bcol(BC_P1 + m), True)
            if p_full is not None:
                ps = ps2_tile(f"ps_p2_{nm}")
                mm_acc(ps[:], [(s_wp2[:, k * 2 * D:k * 2 * D + 128],
                                r1[:, k * NB:(k + 1) * NB]) for k in range(4)])
                evict(p_full[:], ps[:], bcol(BC_P2F), False)
                if p_logv is not None:
                    # logv copy to partitions 0-63 for the off-critical KL
                    # math: SBUF->SBUF DMA beats 4 extra matmuls
                    nc.sync.dma_start(out=p_logv[:],
                                      in_=p_full[D:128, :].bitcast(F32))
            else:
                psm = ps2_tile(f"ps_p2m_{nm}")
                mm_acc(psm[0:D, :], [(s_wp2[:, k * 2 * D:k * 2 * D + D],
                                      r1[:, k * NB:(k + 1) * NB]) for k in range(4)])
                evict(p_mean[:], psm[0:D, :], bcol(BC_P2M, hi=D), False)
                psl = ps2_tile(f"ps_p2l_{nm}")
                mm_acc(psl[0:D, :], [(s_wp2[:, k * 2 * D + D:(k + 1) * 2 * D],
                                      r1[:, k * NB:(k + 1) * NB]) for k in range(4)])
                evict(p_logv[:], psl[0:D, :], bcol(BC_P2L, hi=D), False)

        def emit_gru(x_pairs_fn, h_sb, h_new, w_h, b_rz0, b_hn0, b_in0,
                     b_zn0, zrow_ap, nm):
            """Generic GRU cell, feature-major.
            x_pairs_fn(m) -> [(lhsT, rhs)] input-side matmuls for gate tile m
            (m 0-3 r, 4-7 z, 8-11 n)."""
            r_sb = prz.tile([128, 4 * NB], F32, name=f"r_sb_{nm}", tag="r_sb")
            z_sb = prz.tile([128, 4 * NB], F32, name=f"z_sb_{nm}", tag="z_sb")
            zc_sb = prz.tile([128, 4 * NB], F32, name=f"zc_sb_{nm}", tag="zc_sb")

            def hpair(k, m):
                return (w_h[:, k * 3 * H + m * 128:k * 3 * H + m * 128 + 128],
                        h_sb[:, k * NB:(k + 1) * NB])

            zh_sb = psc.tile([128, 4 * NB], F32, name=f"zh_{nm}", tag="zh",
                             bufs=2)
            # Interleave (r,z,n) per m-tile so each blend chain starts while
            # the GRU's remaining matmuls still occupy the PE; h-side first
            # in each accumulation group so late inputs (p_end/gi) are
            # needed as late as possible.
            for mi in range(4):
                blkg = slice(mi * NB, (mi + 1) * NB)
                m = mi
                ps = ps_tile(f"ps_g_{nm}_{m}")
                mm_acc(ps[:], [hpair(k, m) for k in range(4)] + x_pairs_fn(m))
                nc.scalar.activation(r_sb[:, blkg], ps[:],
                                     AF.Sigmoid, bias=bcol(b_rz0 + m))
                m = 4 + mi
                ps = ps_tile(f"ps_g_{nm}_z{m}")
                pairs = [hpair(k, m) for k in range(4)] + x_pairs_fn(m)
                if zrow_ap is not None:
                    pairs.append((s_ones[:], zrow_ap))
                mm_acc(ps[:], pairs)
                nc.scalar.activation(z_sb[:, blkg], ps[:],
                                     AF.Sigmoid, bias=bcol(b_rz0 + m))
                nc.scalar.activation(zc_sb[:, blkg], ps[:], AF.Sigmoid,
                                     bias=bcol(b_zn0 + mi), scale=-1.0)
                nc.gpsimd.tensor_mul(zh_sb[:, blkg], z_sb[:, blkg],
                                     h_sb[:, blkg])
                m = 8 + mi
                ps_h = ps_tile(f"ps_gh_{nm}_{mi}")
                mm_acc(ps_h[:], [hpair(k, m) for k in range(4)])
                ps_i = ps_tile(f"ps_gi_{nm}_{mi}")
                mm_acc(ps_i[:], x_pairs_fn(m))
                blk = blkg
                t1 = psc.tile([128, NB], F32, name=f"t1_{nm}_{mi}", tag="sc")
                nc.vector.scalar_tensor_tensor(
                    out=t1[:], in0=ps_h[:], scalar=bcol(b_hn0 + mi),
                    in1=r_sb[:, blk], op0=ALU.add, op1=ALU.mult)
                t2 = psc.tile([128, NB], F32, name=f"t2_{nm}_{mi}", tag="sc")
                nc.vector.tensor_add(t2[:], t1[:], ps_i[:])
                n_t = pn.tile([128, NB], F32, name=f"n_{nm}_{mi}", tag="nt")
                nc.scalar.activation(n_t[:], t2[:], AF.Tanh, bias=bcol(b_in0 + mi))
                aa = psc.tile([128, NB], F32, name=f"a_{nm}_{mi}", tag="sc")
                eng = nc.vector if mi % 2 == 1 else nc.gpsimd
                eng.tensor_mul(aa[:], zc_sb[:, blk], n_t[:])
                eng.tensor_add(h_new[:, blk], aa[:], zh_sb[:, blk])

        # ================= setup: h0 = covariates_map(cov), p0 =================
        covt = pin.tile([COV, NB], F32R, name="covt_t", tag="cov")
        nc.sync.dma_start(out=covt[:], in_=d_cov[:])
        ps = ps_tile("ps_cov1")
        nc.tensor.matmul(ps[0:COVH, :], s_wcv1[:], covt[:], start=True, stop=True)
        c1 = ps64.tile([COVH, NB], F32R, name="c1_cov", tag="s64")
        nc.vector.tensor_scalar(out=c1[:], in0=ps[0:COVH, :],
                                scalar1=bcol(BC_CV1, hi=COVH), scalar2=0.0,
                                op0=ALU.add, op1=ALU.max)
        for m in range(4):
            ps2 = ps_tile(f"ps_cov2_{m}")
            nc.tensor.matmul(ps2[:], s_wcv2[:, m * 128:(m + 1) * 128],
                             c1[:], start=True, stop=True)
            nc.scalar.activation(h_a[:, m * NB:(m + 1) * NB], ps2[:],
                                 AF.Tanh, bias=bcol(BC_CV2 + m))
        emit_pmodel(h_a, "p0", p_full=p_end)
        pending_kl = [None]

        def emit_kl():
            if pending_kl[0] is None:
                return
            s, p_lv2, mo_t, xm_lo = pending_kl[0]
            pending_kl[0] = None
            ex = ps64.tile([D, NB], F32, name=f"ex_{s}", tag="s64")
            nc.scalar.activation(ex[:], p_lv2[:], AF.Exp)
            j3 = ps64.tile([D, NB], F32, name=f"j3_{s}", tag="s64")
            nc.vector.scalar_tensor_tensor(
                out=j3[:], in0=ex[:], scalar=1.0, in1=mo_t[:],
                op0=ALU.bypass, op1=ALU.mult, accum_out=acol(3, s))
            j4 = ps64.tile([D, NB], F32, name=f"j4_{s}", tag="s64")
            nc.vector.scalar_tensor_tensor(
                out=j4[:], in0=p_lv2[:], scalar=1.0, in1=mo_t[:],
                op0=ALU.bypass, op1=ALU.mult, accum_out=acol(4, s))
            m2m = ps64.tile([D, NB], F32, name=f"m2m_{s}", tag="s64")
            nc.gpsimd.tensor_mul(m2m[:], p_end[0:D, :], mo_t[:])
            dm = ps64.tile([D, NB], F32, name=f"dm_{s}", tag="s64")
            nc.gpsimd.tensor_sub(dm[:], m2m[:], xm_lo[:])
            j5 = ps64.tile([D, NB], F32, name=f"j5_{s}", tag="s64")
            nc.vector.scalar_tensor_tensor(
                out=j5[:], in0=dm[:], scalar=1.0, in1=dm[:],
                op0=ALU.bypass, op1=ALU.mult, accum_out=acol(5, s))

        # ================= the steps =================
        for s in range(n_steps):
            # --- continuous GRU: h_a -> h_b, input p_end ---
            def cont_x(m, _pe=p_end):
                return [(s_wci[:, m * 128:(m + 1) * 128], _pe[:])]
            emit_gru(cont_x, h_a, h_b, s_wch,
                     BC_CONT_RZ, BC_CONT_HN, BC_CONT_IN, BC_CONT_ZN,
                     None, f"c{s}")
            p_mean = pst.tile([D, NB], F32, name=f"pm_{s}", tag="pmean")
            p_logv = pst.tile([D, NB], F32, name=f"pl_{s}", tag="plogv")
            emit_pmodel(h_b, f"m{s}", p_mean=p_mean, p_logv=p_logv)

            # --- inputs for this step ---
            stA = pst.tile([128, NB], F32R, name=f"stA_{s}", tag="stA")
            stB = pst.tile([128, NB], F32R, name=f"stB_{s}", tag="stB")
            stC = pst.tile([D, NB], F32R, name=f"stC_{s}", tag="stC")
            mo_t = pin.tile([D, NB], F32, name=f"mo_{s}", tag="mo")
            xm_lo = pin.tile([D, NB], F32, name=f"xml_{s}", tag="xml")
            zr_t = pin.tile([1, NB], F32R, name=f"zr_{s}", tag="zr")
            nc.sync.dma_start(out=stA[D:128, :], in_=d_xm[s])
            nc.sync.dma_start(out=xm_lo[:], in_=d_xm[s].bitcast(F32))
            nc.sync.dma_start(out=mo_t[:], in_=d_mo[s])
            nc.sync.dma_start(out=stB[D:128, :], in_=d_mo[s].bitcast(F32R))
            nc.sync.dma_start(out=zr_t[:], in_=d_zr[s])

            # --- obs prep: stacked blocks + l1 partials + gi ---
            # stA = [meanM_lo ; XM_hi], stB = [logvM_lo ; Mo_hi], stC = errM
            # (tensor_tensor_reduce rejects f32r operands -> keep loss math
            #  in f32 scratch, cast-copy into the f32r stacked tiles)
            nc.vector.tensor_mul(stA[0:D, :], p_mean[:], mo_t[:])
            # logvM -> stB + l1lv partial in one op
            nc.vector.scalar_tensor_tensor(
                out=stB[0:D, :], in0=p_logv[:], scalar=1.0, in1=mo_t[:],
                op0=ALU.bypass, op1=ALU.mult, accum_out=acol(1, s))
            rs = ps64.tile([D, NB], F32, name=f"rs_{s}", tag="s64")
            nc.scalar.activation(rs[:], p_logv[:], AF.Exp, scale=-0.5)
            dx = ps64.tile([D, NB], F32, name=f"dx_{s}", tag="s64")
            nc.vector.tensor_sub(dx[:], xm_lo[:], stA[0:D, :])
            nc.vector.tensor_mul(stC[:], dx[:], rs[:])
            j0 = ps64.tile([D, NB], F32, name=f"j0_{s}", tag="s64")
            nc.vector.scalar_tensor_tensor(
                out=j0[:], in0=stC[:], scalar=1.0, in1=stC[:],
                op0=ALU.bypass, op1=ALU.mult, accum_out=acol(0, s))
            red(2, s, mo_t[:])
            gi_sb = pgi.tile([128, 5 * NB], F32R, name=f"gi_{s}", tag="gi")
            for mi in range(5):
                psg = ps2_tile(f"ps_aug_{s}_{mi}")
                mm_acc(psg[:], [
                    (s_waug[:, 0 * D * P + mi * 128:0 * D * P + mi * 128 + 128], stA[:]),
                    (s_waug[:, 1 * D * P + mi * 128:1 * D * P + mi * 128 + 128], stB[:]),
                    (s_waug2[:, mi * 128:mi * 128 + 128], stC[:]),
                ])
                evict(gi_sb[:, mi * NB:(mi + 1) * NB], psg[:], None, True)

            emit_kl()  # prev step's KL: Exp next to rs in ACT queue, DVE
            # work lands in the obs-GRU matmul shadow

            # --- observation GRU: h_b -> h_a, input gi ---
            def obs_x(m, _gi=gi_sb):
                return [(s_woi[:, k * 3 * H + m * 128:k * 3 * H + m * 128 + 128],
                         _gi[:, k * NB:(k + 1) * NB]) for k in range(5)]
            emit_gru(obs_x, h_b, h_a, s_woh,
                     BC_OBS_RZ, BC_OBS_HN, BC_OBS_IN, BC_OBS_ZN,
                     zr_t[:], f"o{s}")
            p_lv2 = pst.tile([D, NB], F32, name=f"plv2_{s}", tag="plv2")
            emit_pmodel(h_a, f"e{s}", p_full=p_end, p_logv=p_lv2)

            # KL partials for step s are emitted next to step s+1's rsig so
            # the two ACT Exp ops share one function-set switch (see emit_kl)
            pending_kl[0] = (s, p_lv2, mo_t, xm_lo)

        emit_kl()

        # ================= epilogue =================
        nc.sync.dma_start(out=d_hout[:], in_=h_a[:].bitcast(F32))
        fin = pstate.tile([D, 6], F32, name="fin")
        for j in range(6):
            nc.vector.tensor_reduce(
                out=fin[:, j:j + 1],
                in_=a_acc[:, j * acc_cols:(j + 1) * acc_cols],
                axis=mybir.AxisListType.X, op=ALU.add)
        nc.sync.dma_start(out=d_acc[:], in_=fin[:])

    nc.compile()
    return nc


def _preprocess(inputs, n_steps):
    """Host-side packing -> list of per-core in_maps."""
    X = np.ascontiguousarray(np.asarray(inputs["X"], np.float32)).reshape(S, O, D)[:n_steps]
    M = np.ascontiguousarray(np.asarray(inputs["M"], np.float32)).reshape(S, O, D)[:n_steps]
    idx = np.asarray(inputs["obs_idx"], np.int64).reshape(S, O)[:n_steps]
    XM = X * M

    XMs = np.zeros((n_steps, B, D), np.float32)
    Mos = np.zeros((n_steps, B, D), np.float32)
    vm = np.zeros((n_steps, B), np.float32)
    sidx = np.repeat(np.arange(n_steps), O)
    fidx = idx.reshape(-1)
    XMs[sidx, fidx] = XM.reshape(-1, D)
    Mos[sidx, fidx] = M.reshape(-1, D)
    vm[sidx, fidx] = 1.0
    zrow = BIG * (1.0 - vm)  # [S, B]

    # augmented einsum weight, row blocks [mean, X, logv, maskbias, err]
    w_prep = np.asarray(inputs["w_prep"], np.float32)
    bias_prep = np.asarray(inputs["bias_prep"], np.float32)
    Waug = np.zeros((5 * D, D * P), np.float32)
    dd = np.arange(D)
    blocks = [w_prep[:, 1, :], w_prep[:, 0, :], w_prep[:, 2, :],
              bias_prep, w_prep[:, 3, :]]
    for bi, blk in enumerate(blocks):
        for d in range(D):
            Waug[bi * D + d, d * P:(d + 1) * P] = blk[d]

    bih_c = np.asarray(inputs["gru_bih"], np.float32)
    bhh_c = np.asarray(inputs["gru_bhh"], np.float32)
    bih_o = np.asarray(inputs["grud_bih"], np.float32)
    bhh_o = np.asarray(inputs["grud_bhh"], np.float32)
    p_b2 = np.asarray(inputs["p_b2"], np.float32)
    bias_pack = np.zeros((128, NBIAS), np.float32)
    bias_pack[:, BC_CONT_RZ:BC_CONT_RZ + 8] = \
        (bih_c + bhh_c)[:2 * H].reshape(8, 128).T
    bias_pack[:, BC_CONT_HN:BC_CONT_HN + 4] = bhh_c[2 * H:].reshape(4, 128).T
    bias_pack[:, BC_CONT_IN:BC_CONT_IN + 4] = bih_c[2 * H:].reshape(4, 128).T
    bias_pack[:, BC_OBS_RZ:BC_OBS_RZ + 8] = \
        (bih_o + bhh_o)[:2 * H].reshape(8, 128).T
    bias_pack[:, BC_OBS_HN:BC_OBS_HN + 4] = bhh_o[2 * H:].reshape(4, 128).T
    bias_pack[:, BC_OBS_IN:BC_OBS_IN + 4] = bih_o[2 * H:].reshape(4, 128).T
    bias_pack[:, BC_P1:BC_P1 + 4] = \
        np.asarray(inputs["p_b1"], np.float32).reshape(4, 128).T
    bias_pack[:, BC_CONT_ZN:BC_CONT_ZN + 4] = \
        -(bih_c + bhh_c)[H:2 * H].reshape(4, 128).T
    bias_pack[:, BC_OBS_ZN:BC_OBS_ZN + 4] = \
        -(bih_o + bhh_o)[H:2 * H].reshape(4, 128).T
    bias_pack[0:D, BC_P2M] = p_b2[0:D]
    bias_pack[0:D, BC_P2L] = p_b2[D:]
    bias_pack[:, BC_P2F] = p_b2
    bias_pack[0:COVH, BC_CV1] = np.asarray(inputs["cov_b1"], np.float32)
    bias_pack[:, BC_CV2:BC_CV2 + 4] = \
        np.asarray(inputs["cov_b2"], np.float32).reshape(4, 128).T

    shared = {
        "wci": np.ascontiguousarray(np.asarray(inputs["gru_wih"], np.float32).T),
        "wch": np.ascontiguousarray(np.asarray(inputs["gru_whh"], np.float32).T),
        "woi": np.ascontiguousarray(np.asarray(inputs["grud_wih"], np.float32).T),
        "woh": np.ascontiguousarray(np.asarray(inputs["grud_whh"], np.float32).T),
        "wp1": np.ascontiguousarray(np.asarray(inputs["p_W1"], np.float32)),
        "wp2": np.ascontiguousarray(np.asarray(inputs["p_W2"], np.float32)),
        "wcv1": np.ascontiguousarray(np.asarray(inputs["cov_W1"], np.float32)),
        "wcv2": np.ascontiguousarray(np.asarray(inputs["cov_W2"], np.float32)),
        "waug": Waug,
        "biasp": bias_pack,
        "onesr": np.ones((1, 128), np.float32),
    }
    cov = np.asarray(inputs["cov"], np.float32)
    in_maps = []
    for c in range(NCORE):
        sl = slice(c * BL, (c + 1) * BL)
        im = dict(shared)
        im["xm"] = np.ascontiguousarray(XMs[:, sl, :].transpose(0, 2, 1))
        im["mo"] = np.ascontiguousarray(Mos[:, sl, :].transpose(0, 2, 1))
        im["zr"] = np.ascontiguousarray(zrow[:, sl][:, None, :])
        im["covt"] = np.ascontiguousarray(cov[sl].T)
        in_maps.append(im)
    return in_maps


def _postprocess(results, inputs):
    h_full = np.zeros((B, H), np.float32)
    acc = np.zeros(6, np.float64)
    for c, res in enumerate(results):
        hk = res["h_out"]  # [128, 4*NB]
        hT = hk.reshape(128, 4, NB).transpose(1, 0, 2).reshape(H, NB)
        h_full[c * BL:(c + 1) * BL] = hT.T
        acc += res["acc_out"].astype(np.float64).sum(axis=0)
    l1 = 0.5 * (acc[0] + acc[1] + 2.0 * LOGC * acc[2])
    l2 = ((np.log(OBS_STD) - 0.5) * acc[2] - 0.5 * acc[4]
          + (acc[3] + acc[5]) / (2.0 * OBS_STD ** 2))
    loss = np.float32(l1 + l2)
    cls_W1 = np.asarray(inputs["cls_W1"], np.float32)
    cls_b1 = np.asarray(inputs["cls_b1"], np.float32)
    cls_W2 = np.asarray(inputs["cls_W2"], np.float32)
    cls_b2 = np.asarray(inputs["cls_b2"], np.float32)
    cls = np.maximum(h_full @ cls_W1 + cls_b1, 0.0) @ cls_W2 + cls_b2
    return h_full, loss, cls.astype(np.float32)


def kernel(_n_steps=S, _spmd_kwargs=None, **inputs):
    if _n_steps not in _CACHE:
        _CACHE[_n_steps] = _build(_n_steps)
    nc = _CACHE[_n_steps]
    in_maps = _preprocess(inputs, _n_steps)
    out = run_bass_kernel_spmd(nc, in_maps, core_ids=list(range(NCORE)),
                               **(_spmd_kwargs or {}))
    h_full, loss, cls = _postprocess(out.results, inputs)
    kernel.last_exec_time_ns = out.exec_time_ns
    return h_full, loss, cls
